# revision 32
# baseline (speedup 1.0000x reference)
"""BertBidaf attention-flow kernel for 8 TRN2 NeuronCores — v10 (manual sync).

Sharding: data-parallel over batch (B=16 -> 2 batches per core); weights
replicated.

The device computes the attention-heavy ~98% of FLOPs: the trilinear
similarity matmul (with the c2q / c*c2q contraction terms riding as 128
extra rhs columns P0/P1), the row softmax statistics, and the fused
attention reductions for terms 2+3. The rank-1 projections
(cwc = c@w_c, q2c = b_att@c, c @ (W1 + W4*q2c)), the softmax division,
and the final row masking are host post-processing (~2% of FLOPs) —
this removes the second (row-major) copy of `c` and the whole
q2c/term1 device tail.

This version hand-schedules the five engine streams with explicit
semaphores (6 sems total) instead of the Tile framework: the Tile
vector-clock exit join (every engine waiting ~2x27 proc lanes, ~9us)
collapses to a single output-DMA wait.

Per-batch device graph:
  mm1:  ps[t] [128, 192] (t = 3 c-row tiles) = rank-3 bias matmul
        (q-side biases + both sequence masks + q@W2+b_out on the P
        columns, as 3 host-built contraction rows) + 16 accumulating
        chunk matmuls (stationary = cT chunks following the DMA
        wavefront for batch 0, tile-major for batch 1 so its epilogues
        overlap the remaining matmuls).
  per tile: nrm = -rowmax(s) (DVE); e = exp(s+nrm) (Scalar);
        den = sum(e), t23raw = rowwise e.P (DVE);
        [t23raw | nrm | den] -> outv[b] (one 6KB DMA per batch).
Host post: m = c@w_c - nrm; b_att = softmax(m); q2c = b_att @ c;
        out = c @ (W1 + W4*q2c) + t23raw/den ; masked rows -> -1e12.
"""

import numpy as np
import ml_dtypes

B, C, Q, D = 16, 384, 64, 2048
NCORES = 8
BPC = B // NCORES  # batches per core
NCH = D // 128     # 16 d-chunks
NW = 192           # mm1 rhs width: 64 s-cols + 2x64 P-cols (c2q/c*c2q)
NEG = np.float32(-1e12)
BF16 = ml_dtypes.bfloat16

_cache = {}


def _build_nc():
    import concourse.bass as bass
    import concourse.bacc as bacc
    from concourse import mybir

    f32 = mybir.dt.float32
    bf16 = mybir.dt.bfloat16
    Ax = mybir.AxisListType.X
    Exp = mybir.ActivationFunctionType.Exp
    mul_op = mybir.AluOpType.mult
    add_op = mybir.AluOpType.add
    max_op = mybir.AluOpType.max

    nc = bacc.Bacc("TRN2", target_bir_lowering=False, debug=False)

    cT = nc.declare_dram_parameter("cT", [BPC, 128, NCH, C], bf16,
                                   isOutput=False)
    qwx = nc.declare_dram_parameter("qwx", [BPC, 128, NCH, NW], bf16,
                                    isOutput=False)
    bias2 = nc.declare_dram_parameter("bias2", [3, BPC, NW + C], bf16,
                                      isOutput=False)
    outv = nc.declare_dram_parameter("outv", [BPC, 128, 12], f32,
                                     isOutput=True)

    b2s = nc.alloc_sbuf_tensor("b2s", [3, BPC, NW + C], bf16)
    qws = [nc.alloc_sbuf_tensor(f"qws{b}", [128, NCH, NW], bf16)
           for b in range(BPC)]
    cts = [nc.alloc_sbuf_tensor(f"cts{b}", [128, NCH, C], bf16)
           for b in range(BPC)]
    es = nc.alloc_sbuf_tensor("es", [128, 2 * 3, 64], f32)
    scr = nc.alloc_sbuf_tensor("scr", [128, 2, 64], f32)
    ovs = [nc.alloc_sbuf_tensor(f"ov{b}", [128, 3, 4], f32)
           for b in range(BPC)]
    ps = [[nc.alloc_psum_tensor(f"ps{b}{t}", [128, NW], f32)
           for t in range(3)] for b in range(BPC)]

    # one semaphore per input DMA (completions on a ring are unordered)
    sA = [nc.alloc_semaphore(f"a{i}") for i in range(7)]  # scalar ring
    sS = [nc.alloc_semaphore(f"s{i}") for i in range(4)]  # sync ring
    pesem = nc.alloc_semaphore("pesem")  # PE per-tile accumulation stops
    nsem = nc.alloc_semaphore("nsem")    # DVE nrm writes
    scsem = nc.alloc_semaphore("scsem")  # Scalar exp writes
    dvsem = nc.alloc_semaphore("dvsem")  # DVE per-tile epilogue complete
    xsem = nc.alloc_semaphore("xsem")    # DVE scr write -> reduce (same-eng RAW)
    osem = nc.alloc_semaphore("osem")    # output DMA completions

    TILES = [(b, t) for b in range(BPC) for t in range(3)]

    with nc.Block(name="kern", no_gpsimd_drain=True) as blk:

        @blk.scalar
        def _(eng):
            eng.dma_start(out=b2s[:, :, :], in_=bias2[:, :, :]) \
                .then_inc(sA[0], 16)
            for g in range(4):
                eng.dma_start(out=qws[0][:, 4 * g:4 * g + 4, :],
                              in_=qwx[0, :, 4 * g:4 * g + 4, :]) \
                    .then_inc(sA[1 + g], 16)
            eng.dma_start(out=qws[1][:, :, :], in_=qwx[1, :, :, :]) \
                .then_inc(sA[5], 16)
            eng.dma_start(out=cts[1][:, 8:16, :], in_=cT[1, :, 8:16, :]) \
                .then_inc(sA[6], 16)
            for i, (b, t) in enumerate(TILES):
                eng.wait_ge(pesem, i + 1)
                eng.wait_ge(nsem, i + 1)
                eng.activation(es[:, 3 * b + t, :], ps[b][t][:, 0:64], Exp,
                               bias=ovs[b][:, t, 2:3], scale=1.0) \
                    .then_inc(scsem, 1)

        @blk.sync
        def _(eng):
            eng.dma_start(out=cts[0][:, 0:2, :], in_=cT[0, :, 0:2, :]) \
                .then_inc(sS[0], 16)
            eng.dma_start(out=cts[0][:, 2:8, :], in_=cT[0, :, 2:8, :]) \
                .then_inc(sS[1], 16)
            eng.dma_start(out=cts[0][:, 8:16, :], in_=cT[0, :, 8:16, :]) \
                .then_inc(sS[2], 16)
            eng.dma_start(out=cts[1][:, 0:8, :], in_=cT[1, :, 0:8, :]) \
                .then_inc(sS[3], 16)
            for b in range(BPC):
                eng.wait_ge(dvsem, 3 * (b + 1))
                eng.dma_start(out=outv[b, :, :],
                              in_=ovs[b].rearrange("p a b -> p (a b)")) \
                    .then_inc(osem, 16)
            eng.wait_ge(osem, 32)

        @blk.tensor
        def _(eng):
            eng.wait_ge(sA[0], 16)
            for b in range(BPC):
                for t in range(3):
                    eng.matmul(ps[b][t][:, :],
                               b2s[:, b, NW + 128 * t:NW + 128 * (t + 1)],
                               b2s[:, b, 0:NW], start=True, stop=False)
            # batch 0: chunk-major behind the DMA wavefront
            awaits = {0: sA[1], 4: sA[2], 8: sA[3], 12: sA[4]}
            swaits = {0: sS[0], 2: sS[1], 8: sS[2]}
            for ch in range(NCH):
                if ch in awaits:
                    eng.wait_ge(awaits[ch], 16)
                if ch in swaits:
                    eng.wait_ge(swaits[ch], 16)
                for t in range(3):
                    mm = eng.matmul(ps[0][t][:, :],
                                    cts[0][:, ch, 128 * t:128 * (t + 1)],
                                    qws[0][:, ch, :],
                                    start=False, stop=(ch == NCH - 1))
                    if ch == NCH - 1:
                        mm.then_inc(pesem, 1)
            # batch 1: tile-major so tiles stop early
            eng.wait_ge(sA[5], 16)
            eng.wait_ge(sS[3], 16)
            for t in range(3):
                for ch in range(NCH):
                    if t == 0 and ch == 8:
                        eng.wait_ge(sA[6], 16)
                    mm = eng.matmul(ps[1][t][:, :],
                                    cts[1][:, ch, 128 * t:128 * (t + 1)],
                                    qws[1][:, ch, :],
                                    start=False, stop=(ch == NCH - 1))
                    if ch == NCH - 1:
                        mm.then_inc(pesem, 1)

        @blk.vector
        def _(eng):
            for i, (b, t) in enumerate(TILES):
                eng.wait_ge(pesem, i + 1)
                eng.tensor_reduce(out=ovs[b][:, t, 2:3],
                                  in_=ps[b][t][:, 0:64], axis=Ax,
                                  op=max_op, negate=True).then_inc(nsem, 1)
                eng.wait_ge(scsem, i + 1)
                e = es[:, 3 * b + t, :]
                eng.tensor_reduce(out=ovs[b][:, t, 3:4], in_=e, axis=Ax,
                                  op=add_op)
                e_dup = bass.AP(tensor=e.tensor, offset=e.offset,
                                ap=[e.ap[0], [0, 2], e.ap[1]])
                eng.tensor_tensor(
                    out=scr[:, :, :],
                    in0=ps[b][t][:, 64:192].rearrange("p (j i) -> p j i",
                                                      j=2),
                    in1=e_dup, op=mul_op).then_inc(xsem, 1)
                eng.wait_ge(xsem, i + 1)
                eng.tensor_reduce(out=ovs[b][:, t, 0:2], in_=scr[:, :, :],
                                  axis=Ax, op=add_op).then_inc(dvsem, 1)

    nc.finalize()
    return nc


def _get_nc():
    if "nc" not in _cache:
        _cache["nc"] = _build_nc()
    return _cache["nc"]


def _prep_host(c, q, c_len, q_len, w_c, b_c, w_q, b_q, w_cq, b_cq, W_out,
               b_out):
    """Build per-core device input maps (host-side layout/masking prep)."""
    c = np.asarray(c, np.float32)
    q = np.asarray(q, np.float32)
    c_len = np.asarray(c_len).astype(np.int64)
    q_len = np.asarray(q_len).astype(np.int64)
    w_c = np.asarray(w_c, np.float32)
    w_q = np.asarray(w_q, np.float32)
    w_cq = np.asarray(w_cq, np.float32)
    W_out = np.asarray(W_out, np.float32)
    b_out = np.asarray(b_out, np.float32)
    b_sum = float(np.asarray(b_c, np.float32) + np.asarray(b_q, np.float32)
                  + np.asarray(b_cq, np.float32))

    Mv = np.float32(BF16(-1e12))
    iq = np.arange(Q)
    W2 = W_out[D:2 * D]       # [D, 2] (x = [c, c2q, c*c2q, c*q2c])
    W3 = W_out[2 * D:3 * D]

    in_maps = []
    for core in range(NCORES):
        bs = [BPC * core + i for i in range(BPC)]
        cTm = np.empty((BPC, 128, NCH, C), BF16)
        qwxm = np.empty((BPC, 128, NCH, NW), BF16)
        b2 = np.zeros((3, BPC, NW + C), BF16)
        for i, bidx in enumerate(bs):
            cTm[i] = c[bidx].T.reshape(NCH, 128, C).transpose(1, 0, 2) \
                .astype(BF16)
            qb = q[bidx]
            qT = qb.T                             # [D, Q]
            blk = np.empty((D, NW), np.float32)
            blk[:, 0:64] = qT * w_cq[:, None]
            blk[:, 64:128] = qT * W3[:, 0:1]
            blk[:, 128:192] = qT * W3[:, 1:2]
            qwxm[i] = blk.reshape(NCH, 128, NW).transpose(1, 0, 2) \
                .astype(BF16)
            qs = qb @ w_q + b_sum
            low = np.where(iq >= q_len[bidx], Mv, np.float32(0))
            hi = np.where((iq < Q - 1) | (iq >= q_len[bidx]), Mv,
                          np.float32(0))
            QW2b = qb @ W2 + b_out[None, :]
            b2[0, i, 0:64] = qs.astype(BF16)
            b2[0, i, 64:128] = QW2b[:, 0].astype(BF16)
            b2[0, i, 128:192] = QW2b[:, 1].astype(BF16)
            b2[1, i, 0:64] = low.astype(BF16)
            b2[2, i, 0:64] = (hi - low).astype(BF16)
            b2[0, i, NW:NW + C] = BF16(1)
            b2[1, i, NW:NW + C] = BF16(1)
            b2[2, i, NW:NW + C] = (np.arange(C) >= c_len[bidx]) \
                .astype(np.float32).astype(BF16)
        in_maps.append(dict(cT=cTm, qwx=qwxm, bias2=b2))
    return in_maps, (c, c_len, W_out, w_c)


def kernel(**inputs):
    from concourse.bass_utils import run_bass_kernel_spmd

    nc = _get_nc()
    in_maps, (c, c_len, W_out, w_c) = _prep_host(**inputs)
    res = run_bass_kernel_spmd(nc, in_maps, core_ids=list(range(NCORES)))
    _cache["last_results"] = res

    W1 = W_out[0:D]          # [D, 2]
    W4 = W_out[3 * D:4 * D]

    out = np.empty((B, C, 2), np.float32)
    for core in range(NCORES):
        o = res.results[core]["outv"].reshape(BPC, 128, 3, 4)
        for i in range(BPC):
            bidx = BPC * core + i
            den = o[i, :, :, 3].T.reshape(C)
            t23 = o[i, :, :, 0:2].transpose(1, 0, 2).reshape(C, 2) \
                / den[:, None]
            nrm = o[i, :, :, 2].T.reshape(C)
            m = c[bidx] @ w_c - nrm
            eb = np.exp(m - m.max())
            b_att = (eb / eb.sum()).astype(np.float32)
            q2c = b_att @ c[bidx]                       # [D]
            w14 = W1 + W4 * q2c[:, None]                # [D, 2]
            out[bidx] = c[bidx] @ w14 + t23

    rows = np.arange(C)[None, :]
    row_mask = (rows >= c_len[:, None]) & (rows < C - 1)
    out0 = np.where(row_mask, NEG, out[..., 0])
    out1 = np.where(row_mask, NEG, out[..., 1])
    return out0, out1


# revision 34
# speedup vs baseline: 1.1689x; 1.1689x over previous
"""BertBidaf attention-flow kernel for 8 TRN2 NeuronCores — v10 (manual sync).

Sharding: data-parallel over batch (B=16 -> 2 batches per core); weights
replicated.

The device computes the attention-heavy ~98% of FLOPs: the trilinear
similarity matmul (with the c2q / c*c2q contraction terms riding as 128
extra rhs columns P0/P1), the row softmax statistics, and the fused
attention reductions for terms 2+3. The rank-1 projections
(cwc = c@w_c, q2c = b_att@c, c @ (W1 + W4*q2c)), the softmax division,
and the final row masking are host post-processing (~2% of FLOPs) —
this removes the second (row-major) copy of `c` and the whole
q2c/term1 device tail.

This version hand-schedules the five engine streams with explicit
semaphores (6 sems total) instead of the Tile framework: the Tile
vector-clock exit join (every engine waiting ~2x27 proc lanes, ~9us)
collapses to a single output-DMA wait.

Per-batch device graph:
  mm1:  ps[t] [128, 192] (t = 3 c-row tiles) = rank-3 bias matmul
        (q-side biases + both sequence masks + q@W2+b_out on the P
        columns, as 3 host-built contraction rows) + 16 accumulating
        chunk matmuls (stationary = cT chunks following the DMA
        wavefront for batch 0, tile-major for batch 1 so its epilogues
        overlap the remaining matmuls).
  per tile: nrm = -rowmax(s) (DVE); e = exp(s+nrm) (Scalar);
        den = sum(e), t23raw = rowwise e.P (DVE);
        [t23raw | nrm | den] -> outv[b] (one 6KB DMA per batch).
Host post: m = c@w_c - nrm; b_att = softmax(m); q2c = b_att @ c;
        out = c @ (W1 + W4*q2c) + t23raw/den ; masked rows -> -1e12.
"""

import numpy as np
import ml_dtypes

B, C, Q, D = 16, 384, 64, 2048
NCORES = 8
BPC = B // NCORES  # batches per core
NCH = D // 128     # 16 d-chunks
NW = 192           # mm1 rhs width: 64 s-cols + 2x64 P-cols (c2q/c*c2q)
NEG = np.float32(-1e12)
BF16 = ml_dtypes.bfloat16

_cache = {}


def _build_nc():
    import concourse.bass as bass
    import concourse.bacc as bacc
    from concourse import mybir

    f32 = mybir.dt.float32
    bf16 = mybir.dt.bfloat16
    Ax = mybir.AxisListType.X
    Exp = mybir.ActivationFunctionType.Exp
    mul_op = mybir.AluOpType.mult
    add_op = mybir.AluOpType.add
    max_op = mybir.AluOpType.max

    nc = bacc.Bacc("TRN2", target_bir_lowering=False, debug=False)

    cT = nc.declare_dram_parameter("cT", [BPC, 128, NCH, C], bf16,
                                   isOutput=False)
    qwx = nc.declare_dram_parameter("qwx", [BPC, 128, NCH, NW], bf16,
                                    isOutput=False)
    bias2 = nc.declare_dram_parameter("bias2", [3, BPC, NW + C], bf16,
                                      isOutput=False)
    outv = nc.declare_dram_parameter("outv", [BPC, 128, 12], f32,
                                     isOutput=True)

    b2s = nc.alloc_sbuf_tensor("b2s", [3, BPC, NW + C], bf16)
    qws = [nc.alloc_sbuf_tensor(f"qws{b}", [128, NCH, NW], bf16)
           for b in range(BPC)]
    cts = [nc.alloc_sbuf_tensor(f"cts{b}", [128, NCH, C], bf16)
           for b in range(BPC)]
    es = nc.alloc_sbuf_tensor("es", [128, 2 * 3, 64], f32)
    scr = nc.alloc_sbuf_tensor("scr", [128, 6, 2, 64], f32)
    ovs = [nc.alloc_sbuf_tensor(f"ov{b}", [128, 3, 4], f32)
           for b in range(BPC)]
    ps = [[nc.alloc_psum_tensor(f"ps{b}{t}", [128, NW], f32)
           for t in range(3)] for b in range(BPC)]

    # one semaphore per input DMA (completions on a ring are unordered)
    sA = [nc.alloc_semaphore(f"a{i}") for i in range(7)]  # scalar ring
    sS = [nc.alloc_semaphore(f"s{i}") for i in range(4)]  # sync ring
    pesem = nc.alloc_semaphore("pesem")  # PE per-tile accumulation stops
    nsem = nc.alloc_semaphore("nsem")    # DVE nrm writes
    scsem = nc.alloc_semaphore("scsem")  # Scalar exp writes
    dvsem = nc.alloc_semaphore("dvsem")  # DVE per-tile epilogue complete
    xsem = nc.alloc_semaphore("xsem")    # DVE scr write -> reduce (same-eng RAW)
    osem = nc.alloc_semaphore("osem")    # output DMA completions

    TILES = [(b, t) for b in range(BPC) for t in range(3)]

    with nc.Block(name="kern", no_gpsimd_drain=True) as blk:

        @blk.scalar
        def _(eng):
            eng.dma_start(out=b2s[:, :, :], in_=bias2[:, :, :]) \
                .then_inc(sA[0], 16)
            for g in range(4):
                eng.dma_start(out=qws[0][:, 4 * g:4 * g + 4, :],
                              in_=qwx[0, :, 4 * g:4 * g + 4, :]) \
                    .then_inc(sA[1 + g], 16)
            eng.dma_start(out=qws[1][:, :, :], in_=qwx[1, :, :, :]) \
                .then_inc(sA[5], 16)
            eng.dma_start(out=cts[1][:, 8:16, :], in_=cT[1, :, 8:16, :]) \
                .then_inc(sA[6], 16)
            for i, (b, t) in enumerate(TILES):
                eng.wait_ge(pesem, i + 1)
                eng.wait_ge(nsem, i + 1)
                eng.activation(es[:, 3 * b + t, :], ps[b][t][:, 0:64], Exp,
                               bias=ovs[b][:, t, 2:3], scale=1.0) \
                    .then_inc(scsem, 1)

        @blk.sync
        def _(eng):
            eng.dma_start(out=cts[0][:, 0:2, :], in_=cT[0, :, 0:2, :]) \
                .then_inc(sS[0], 16)
            eng.dma_start(out=cts[0][:, 2:8, :], in_=cT[0, :, 2:8, :]) \
                .then_inc(sS[1], 16)
            eng.dma_start(out=cts[0][:, 8:16, :], in_=cT[0, :, 8:16, :]) \
                .then_inc(sS[2], 16)
            eng.dma_start(out=cts[1][:, 0:8, :], in_=cT[1, :, 0:8, :]) \
                .then_inc(sS[3], 16)
            for b in range(BPC):
                eng.wait_ge(dvsem, 3 * (b + 1))
                eng.dma_start(out=outv[b, :, :],
                              in_=ovs[b].rearrange("p a b -> p (a b)")) \
                    .then_inc(osem, 16)
            eng.wait_ge(osem, 32)

        @blk.tensor
        def _(eng):
            eng.wait_ge(sA[0], 16)
            for b in range(BPC):
                for t in range(3):
                    eng.matmul(ps[b][t][:, :],
                               b2s[:, b, NW + 128 * t:NW + 128 * (t + 1)],
                               b2s[:, b, 0:NW], start=True, stop=False)
            # batch 0: chunk-major behind the DMA wavefront
            awaits = {0: sA[1], 4: sA[2], 8: sA[3], 12: sA[4]}
            swaits = {0: sS[0], 2: sS[1], 8: sS[2]}
            for ch in range(NCH):
                if ch in awaits:
                    eng.wait_ge(awaits[ch], 16)
                if ch in swaits:
                    eng.wait_ge(swaits[ch], 16)
                for t in range(3):
                    mm = eng.matmul(ps[0][t][:, :],
                                    cts[0][:, ch, 128 * t:128 * (t + 1)],
                                    qws[0][:, ch, :],
                                    start=False, stop=(ch == NCH - 1))
                    if ch == NCH - 1:
                        mm.then_inc(pesem, 1)
            # batch 1: tile-major so tiles stop early
            eng.wait_ge(sA[5], 16)
            eng.wait_ge(sS[3], 16)
            for t in range(3):
                for ch in range(NCH):
                    if t == 0 and ch == 8:
                        eng.wait_ge(sA[6], 16)
                    mm = eng.matmul(ps[1][t][:, :],
                                    cts[1][:, ch, 128 * t:128 * (t + 1)],
                                    qws[1][:, ch, :],
                                    start=False, stop=(ch == NCH - 1))
                    if ch == NCH - 1:
                        mm.then_inc(pesem, 1)

        @blk.vector
        def _(eng):
            # software-pipelined: tile i+1's nrm issues before tile i's
            # heavy ops so the Scalar exp overlaps the DVE reductions
            def nrm(i):
                b, t = TILES[i]
                eng.wait_ge(pesem, i + 1)
                eng.tensor_reduce(out=ovs[b][:, t, 2:3],
                                  in_=ps[b][t][:, 0:64], axis=Ax,
                                  op=max_op, negate=True).then_inc(nsem, 1)

            def body(i):
                b, t = TILES[i]
                eng.wait_ge(scsem, i + 1)
                e = es[:, 3 * b + t, :]
                eng.tensor_reduce(out=ovs[b][:, t, 3:4], in_=e, axis=Ax,
                                  op=add_op)
                e_dup = bass.AP(tensor=e.tensor, offset=e.offset,
                                ap=[e.ap[0], [0, 2], e.ap[1]])
                eng.tensor_tensor(
                    out=scr[:, i, :, :],
                    in0=ps[b][t][:, 64:192].rearrange("p (j i) -> p j i",
                                                      j=2),
                    in1=e_dup, op=mul_op).then_inc(xsem, 1)
                eng.wait_ge(xsem, i + 1)
                eng.tensor_reduce(out=ovs[b][:, t, 0:2],
                                  in_=scr[:, i, :, :],
                                  axis=Ax, op=add_op).then_inc(dvsem, 1)

            nrm(0)
            for i in range(1, 6):
                nrm(i)
                body(i - 1)
            body(5)

    nc.finalize()
    return nc


def _get_nc():
    if "nc" not in _cache:
        _cache["nc"] = _build_nc()
    return _cache["nc"]


def _prep_host(c, q, c_len, q_len, w_c, b_c, w_q, b_q, w_cq, b_cq, W_out,
               b_out):
    """Build per-core device input maps (host-side layout/masking prep)."""
    c = np.asarray(c, np.float32)
    q = np.asarray(q, np.float32)
    c_len = np.asarray(c_len).astype(np.int64)
    q_len = np.asarray(q_len).astype(np.int64)
    w_c = np.asarray(w_c, np.float32)
    w_q = np.asarray(w_q, np.float32)
    w_cq = np.asarray(w_cq, np.float32)
    W_out = np.asarray(W_out, np.float32)
    b_out = np.asarray(b_out, np.float32)
    b_sum = float(np.asarray(b_c, np.float32) + np.asarray(b_q, np.float32)
                  + np.asarray(b_cq, np.float32))

    Mv = np.float32(BF16(-1e12))
    iq = np.arange(Q)
    W2 = W_out[D:2 * D]       # [D, 2] (x = [c, c2q, c*c2q, c*q2c])
    W3 = W_out[2 * D:3 * D]

    in_maps = []
    for core in range(NCORES):
        bs = [BPC * core + i for i in range(BPC)]
        cTm = np.empty((BPC, 128, NCH, C), BF16)
        qwxm = np.empty((BPC, 128, NCH, NW), BF16)
        b2 = np.zeros((3, BPC, NW + C), BF16)
        for i, bidx in enumerate(bs):
            cTm[i] = c[bidx].T.reshape(NCH, 128, C).transpose(1, 0, 2) \
                .astype(BF16)
            qb = q[bidx]
            qT = qb.T                             # [D, Q]
            blk = np.empty((D, NW), np.float32)
            blk[:, 0:64] = qT * w_cq[:, None]
            blk[:, 64:128] = qT * W3[:, 0:1]
            blk[:, 128:192] = qT * W3[:, 1:2]
            qwxm[i] = blk.reshape(NCH, 128, NW).transpose(1, 0, 2) \
                .astype(BF16)
            qs = qb @ w_q + b_sum
            low = np.where(iq >= q_len[bidx], Mv, np.float32(0))
            hi = np.where((iq < Q - 1) | (iq >= q_len[bidx]), Mv,
                          np.float32(0))
            QW2b = qb @ W2 + b_out[None, :]
            b2[0, i, 0:64] = qs.astype(BF16)
            b2[0, i, 64:128] = QW2b[:, 0].astype(BF16)
            b2[0, i, 128:192] = QW2b[:, 1].astype(BF16)
            b2[1, i, 0:64] = low.astype(BF16)
            b2[2, i, 0:64] = (hi - low).astype(BF16)
            b2[0, i, NW:NW + C] = BF16(1)
            b2[1, i, NW:NW + C] = BF16(1)
            b2[2, i, NW:NW + C] = (np.arange(C) >= c_len[bidx]) \
                .astype(np.float32).astype(BF16)
        in_maps.append(dict(cT=cTm, qwx=qwxm, bias2=b2))
    return in_maps, (c, c_len, W_out, w_c)


def kernel(**inputs):
    from concourse.bass_utils import run_bass_kernel_spmd

    nc = _get_nc()
    in_maps, (c, c_len, W_out, w_c) = _prep_host(**inputs)
    res = run_bass_kernel_spmd(nc, in_maps, core_ids=list(range(NCORES)))
    _cache["last_results"] = res

    W1 = W_out[0:D]          # [D, 2]
    W4 = W_out[3 * D:4 * D]

    out = np.empty((B, C, 2), np.float32)
    for core in range(NCORES):
        o = res.results[core]["outv"].reshape(BPC, 128, 3, 4)
        for i in range(BPC):
            bidx = BPC * core + i
            den = o[i, :, :, 3].T.reshape(C)
            t23 = o[i, :, :, 0:2].transpose(1, 0, 2).reshape(C, 2) \
                / den[:, None]
            nrm = o[i, :, :, 2].T.reshape(C)
            m = c[bidx] @ w_c - nrm
            eb = np.exp(m - m.max())
            b_att = (eb / eb.sum()).astype(np.float32)
            q2c = b_att @ c[bidx]                       # [D]
            w14 = W1 + W4 * q2c[:, None]                # [D, 2]
            out[bidx] = c[bidx] @ w14 + t23

    rows = np.arange(C)[None, :]
    row_mask = (rows >= c_len[:, None]) & (rows < C - 1)
    out0 = np.where(row_mask, NEG, out[..., 0])
    out1 = np.where(row_mask, NEG, out[..., 1])
    return out0, out1
